# revision 1
# baseline (speedup 1.0000x reference)
"""Bass/Trainium2 kernel for the DisentangleLoss (NT-Xent style contrastive loss).

Math (matches the reference):
    sn = s / max(||s||, eps)                     row-normalized embeddings
    sim = (sn @ sn.T) / TEMP                     [K, K] similarity logits
    positives of row i: columns j != i with j ≡ i (mod BS)   (8 per row)
    negatives of row i: everything else except the diagonal  (K-9 per row)
    loss = mean over (row, positive) of  logaddexp(p, lse(negatives)) - p

Device strategy (8 NeuronCores, SPMD):
  * Each core gets a row-rolled copy of s (np.roll by -1152*c) and computes the
    loss terms for its local rows 0..1151.  Rolling preserves residues mod BS
    (K = 9*BS), so positives for local row i sit at columns i%1024 + 1024*m —
    the same offsets on every core -> a single uniform program.
  * The [1152, 9216] block of sim is produced in [128, 2048] PSUM groups
    (bf16 matmuls of sqrt(1/TEMP)-scaled normalized embeddings; rsqrt for the
    normalization is computed as exp(-0.5*ln(ss)) so the whole kernel uses a
    single ACT table set).  Each group is evacuated from PSUM by either the
    scalar engine (Exp with accum_out producing the row sum, result to SBUF
    bf16) or, for OFFLOAD_GK groups, the vector engine (Schraudolph bitcast
    exp + reduce) — balancing the two engines that can read PSUM.  The 9
    "diagonal" subtiles per row-tile (self + 8 positives) are extracted with
    fused multiply-reduce ops against an identity mask on the vector engine;
    normalize/transpose prologue work is emitted interleaved per input chunk
    (engines execute in FIFO order), with scale-muls on GPSIMD and the
    [D, K] transpose done on the tensor engine through shared PSUM slots.
  * negsum = rowtotal - sum(exp(diag entries)); the self term is removed via
    the max over the 9 entries (self-similarity == 1 is always the max).
    loss terms use log(e_p + negsum) - log(e_p) with a single batched Ln.
  * Each core writes a [128, 1] partial (per-partition loss sums); the host
    adds them up and divides by K*(N-1).
"""

import math

import numpy as np

K = 9216
D = 128
BS = 1024
N = 9
TEMP = 0.5
NCORES = 8
R = K // NCORES          # 1152 rows per core
RT = R // 128            # 9 row tiles per core
CT = K // 128            # 72 column tiles
KPOS = K * (N - 1)

# Offload column-group 4's exp+rowsum to the vector engine via a Schraudolph
# bit-trick exp (exp(x) ~= bitcast_f32(int32(x*2^23/ln2 + B))).  ACT is the
# bottleneck engine (~2us per 2048-wide exp); DVE has slack.  B is fitted so
# the mean relative error over the actual logit distribution is ~0 (max 3.3%
# per element, which averages out in the 9207-term logsumexp).
USE_DVE_EXP = True
# (An int16/bf16 variant of this trick -- 2-byte outputs enabling the DVE 2x
# perf mode on the following reduce -- measured fine numerically but crashed
# the device intermittently (NRT_EXEC_UNIT_UNRECOVERABLE on 1 of 5 runs);
# the int32 form below has been reliable across 25+ hardware executions.)
SCHRAUDOLPH_S = 12102203.0        # 2^23/ln2, exactly representable in f32
SCHRAUDOLPH_B = 1064951741.0
# (col-group, row-tile) pairs offloaded to the DVE exp, spread evenly over
# the schedule to balance ACT (~73us) vs DVE (~73us) busy time end-to-end.
# Never offload g=0: the self-similarity entries live there and the
# max-based self detection plus the d2 loss term want them exact.
OFFLOAD_GK = ({(g, k) for g in (1, 2, 3) for k in (2, 6)}
              | {(4, 1), (4, 4), (4, 7)})

_CACHE = {}


def _build():
    import concourse.bacc as bacc
    import concourse.tile as tile
    from concourse import mybir
    from concourse.masks import make_identity

    # Steer the ACT-table placement pass: every Exp/Ln in this kernel should
    # be served by the one set containing both ("natural_log_exp_and_others"),
    # otherwise the per-func first-match choice alternates tables and inserts
    # a ~2.7us ACT_TABLE_LOAD per switch.  Indices (= act_func_set_id) of the
    # remaining sets are preserved; only their advertised contents shrink.
    if not getattr(bacc, "_ant_act_tables_patched", False):
        _orig_get_tables = bacc.get_activation_tables

        def _patched_get_tables(arch):
            tables = dict(_orig_get_tables(arch))
            exp_ln = {mybir.ActivationFunctionType.Exp,
                      mybir.ActivationFunctionType.Ln}
            for name, funcs in tables.items():
                if name != "natural_log_exp_and_others" and \
                        exp_ln <= tables.get("natural_log_exp_and_others",
                                             set()):
                    tables[name] = funcs - exp_ln
            return tables

        bacc.get_activation_tables = _patched_get_tables
        bacc._ant_act_tables_patched = True

    f32 = mybir.dt.float32
    bf16 = mybir.dt.bfloat16
    AF = mybir.ActivationFunctionType
    OP = mybir.AluOpType
    AX = mybir.AxisListType

    nc = bacc.Bacc("TRN2", target_bir_lowering=False, debug=False,
                   num_devices=NCORES)
    s_in = nc.dram_tensor("s", [K, D], f32, kind="ExternalInput")
    y_out = nc.dram_tensor("part", [128, 1], f32, kind="ExternalOutput")

    with tile.TileContext(nc) as tc:
        with (
            tc.tile_pool(name="big", bufs=1) as big,
            tc.tile_pool(name="small", bufs=1) as small,
            tc.tile_pool(name="scr", bufs=4) as scr_pool,
            tc.tile_pool(name="ex", bufs=4) as ex_pool,
            tc.tile_pool(name="psum", bufs=2, space="PSUM") as pp,
        ):
            s_rows = big.tile([128, CT * 128], f32)    # raw rows, partition=row%128
            sn_rows = big.tile([128, CT * 128], bf16)  # normalized+scaled rows
            snT = big.tile([128, CT * 128], bf16)      # [D, K] transposed
            ident = small.tile([128, 128], f32)
            make_identity(nc, ident)
            ident_bf = small.tile([128, 128], bf16)
            nc.vector.tensor_copy(ident_bf[:], ident[:])

            ss = small.tile([128, CT], f32)       # per-row sum of squares
            lnss = small.tile([128, CT], f32)
            sclr = small.tile([128, CT], f32)
            scl = small.tile([128, CT], f32)      # sqrt(1/TEMP)/max(norm,eps)
            tot5 = small.tile([128, RT * 5], f32)  # exp row sums per col-group
            epos = small.tile([128, RT * 9], f32)  # exp(diag entries)

            # ---- prologue building blocks ----
            # Loads are issued up front (they stream on SWDGE queues); the
            # per-chunk normalize work is emitted interleaved with the main
            # loop below because ACT/DVE execute in FIFO order -- emitting
            # all prologue work first would stall the main loop behind the
            # last chunk's load.
            for ch in range(9):
                src = s_in[ch * 1024:(ch + 1) * 1024, :].rearrange(
                    "(t p) d -> p t d", p=128)
                dst = s_rows[:, ch * 1024:(ch + 1) * 1024].rearrange(
                    "p (t d) -> p t d", d=128)
                nc.gpsimd.dma_start(out=dst, in_=src)

            # 1/max(norm,eps)*sqrt(1/TEMP) computed as exp(-0.5*ln(ss))*sqrt2
            # (clamped) -- keeps every ACT instruction in the same table set
            # (natural_log_exp) as the main-loop exps: no table reloads.
            rt2 = math.sqrt(1.0 / TEMP)
            bias_t = small.tile([128, 1], f32)
            nc.vector.memset(bias_t, math.log(rt2))

            def normalize_chunks(chunks):
                # sumsq per row tile, rsqrt via ln/exp, scale+cast to bf16
                for ch in chunks:
                    for k in range(ch * 8, ch * 8 + 8):
                        sl = slice(k * 128, (k + 1) * 128)
                        sc = scr_pool.tile([128, 128], f32, tag="ssq")
                        nc.vector.scalar_tensor_tensor(
                            out=sc, in0=s_rows[:, sl], scalar=1.0,
                            in1=s_rows[:, sl], op0=OP.mult, op1=OP.mult,
                            accum_out=ss[:, k:k + 1])
                gsl = slice(chunks[0] * 8, (chunks[-1] + 1) * 8)
                nc.scalar.activation(out=lnss[:, gsl], in_=ss[:, gsl],
                                     func=AF.Ln)
                nc.scalar.activation(out=sclr[:, gsl], in_=lnss[:, gsl],
                                     func=AF.Exp, scale=-0.5,
                                     bias=bias_t[:])
                nc.vector.tensor_scalar_min(scl[:, gsl], sclr[:, gsl],
                                            rt2 * 1e8)
                # scale+cast on GPSIMD (idle once its load dma_starts have
                # generated descriptors) -- takes ~11us off the vector
                # engine.  The first two chunks go on DVE so the matmul
                # pipeline starts without waiting on Pool's load descgen.
                for ch in chunks:
                    smul_eng = nc.vector if ch < 2 else nc.gpsimd
                    for k in range(ch * 8, ch * 8 + 8):
                        sl = slice(k * 128, (k + 1) * 128)
                        smul_eng.tensor_scalar_mul(sn_rows[:, sl],
                                                   s_rows[:, sl],
                                                   scl[:, k:k + 1])

            def transpose_batch(b):
                # PE-transpose 16 row-tiles (= cols b*2048 .. +2048 of snT)
                # through one psum tile, evacuate with one DVE copy.
                n = min(16, CT - b * 16)
                pt = pp.tile([128, 2048], bf16, tag="pg")
                for t in range(n):
                    k = b * 16 + t
                    nc.tensor.transpose(
                        pt[:, t * 128:(t + 1) * 128],
                        sn_rows[:, k * 128:(k + 1) * 128], ident_bf[:])
                nc.vector.tensor_copy(
                    snT[:, b * 2048:b * 2048 + n * 128], pt[:, :n * 128])

            # ---- main loop: sim row-block -> exp -> row sums + diagonals ----
            # column-group outer, with the PE transposes producing each snT
            # 2048-chunk emitted just-in-time before the group needing them
            # (PE executes in FIFO order; this starts group 0 as soon as the
            # first two input chunks are loaded+normalized).
            def emit_mms(g, k):
                lhsT = snT[:, k * 128:(k + 1) * 128]
                width = 2048 if g < 4 else 1024
                pg = pp.tile([128, 2048], f32, tag="pg")
                for j in range(width // 512):
                    col = g * 2048 + j * 512
                    nc.tensor.matmul(
                        pg[:, j * 512:(j + 1) * 512], lhsT,
                        snT[:, col:col + 512], start=True, stop=True)
                return pg

            def emit_evac(pg, g, k):
                c0 = (128 * k) % BS
                width = 2048 if g < 4 else 1024
                ms = [2 * g, 2 * g + 1] if g < 4 else [8]
                if USE_DVE_EXP and (g, k) in OFFLOAD_GK:
                    q = scr_pool.tile([128, 2048], mybir.dt.int32,
                                      tag="qexp")
                    nc.vector.tensor_scalar(
                        out=q[:, :width], in0=pg[:, :width],
                        scalar1=SCHRAUDOLPH_S, scalar2=SCHRAUDOLPH_B,
                        op0=OP.mult, op1=OP.add)
                    nc.vector.reduce_sum(
                        out=tot5[:, k * 5 + g:k * 5 + g + 1],
                        in_=q[:, :width].bitcast(f32), axis=AX.X)
                    for m in ms:
                        off = c0 + 1024 * m - 2048 * g
                        dsc = scr_pool.tile([128, 128], f32, tag="diag")
                        nc.vector.scalar_tensor_tensor(
                            out=dsc, in0=q[:, off:off + 128].bitcast(f32),
                            scalar=1.0, in1=ident, op0=OP.mult, op1=OP.mult,
                            accum_out=epos[:, k * 9 + m:k * 9 + m + 1])
                    return
                ex = ex_pool.tile([128, 2048], bf16, tag="ex")
                nc.scalar.activation(
                    out=ex[:, :width], in_=pg[:, :width], func=AF.Exp,
                    accum_out=tot5[:, k * 5 + g:k * 5 + g + 1])
                for m in ms:
                    off = c0 + 1024 * m - 2048 * g
                    # all-bf16 non-scalar APs -> DVE 2x_1p perf mode
                    dsc = scr_pool.tile([128, 128], bf16, tag="diag")
                    nc.vector.scalar_tensor_tensor(
                        out=dsc, in0=ex[:, off:off + 128], scalar=1.0,
                        in1=ident_bf, op0=OP.mult, op1=OP.mult,
                        accum_out=epos[:, k * 9 + m:k * 9 + m + 1])

            # Software-pipelined EMISSION: group j+1's matmuls are emitted
            # before group j's evacuation so the Tile scheduler (which
            # prioritizes by emission order) overlaps them.
            pending = None
            for g in range(5):
                normalize_chunks([2 * g, 2 * g + 1] if g < 4 else [8])
                transpose_batch(g)
                for k in range(RT):
                    pg = emit_mms(g, k)
                    if pending is not None:
                        emit_evac(*pending)
                    pending = (pg, g, k)
            emit_evac(*pending)

            # ---- final phase: negsum, max trick, batched Ln, partials ----
            rowtot = small.tile([128, RT], f32)
            nc.vector.reduce_sum(
                out=rowtot, in_=tot5[:].rearrange("p (k g) -> p k g", g=5),
                axis=AX.X)
            sumep = small.tile([128, RT], f32)
            nc.vector.reduce_sum(
                out=sumep, in_=epos[:].rearrange("p (k m) -> p k m", m=9),
                axis=AX.X)
            negsum = small.tile([128, RT], f32)
            nc.vector.tensor_sub(negsum, rowtot, sumep)
            emax = small.tile([128, RT], f32)
            nc.vector.reduce_max(
                out=emax, in_=epos[:].rearrange("p (k m) -> p k m", m=9),
                axis=AX.X)

            NP9 = RT * 9  # 81
            lnin = small.tile([128, 2 * NP9 + 2 * RT], f32)
            for k in range(RT):
                nc.vector.tensor_scalar_add(
                    lnin[:, k * 9:(k + 1) * 9], epos[:, k * 9:(k + 1) * 9],
                    negsum[:, k:k + 1])
            nc.vector.tensor_add(lnin[:, NP9:NP9 + RT], emax, negsum)
            nc.vector.tensor_copy(lnin[:, NP9 + RT:2 * NP9 + RT], epos[:])
            nc.vector.tensor_copy(lnin[:, 2 * NP9 + RT:2 * NP9 + 2 * RT],
                                  emax[:])
            lnout = small.tile([128, 2 * NP9 + 2 * RT], f32)
            nc.scalar.activation(out=lnout, in_=lnin, func=AF.Ln)

            # loss partial per partition: sum(ln(e+negsum)-ln(e)) terms,
            # minus the self terms (identified via the max).  The sub+reduce
            # pairs fuse into single scalar_tensor_tensor ops (accum_out).
            d1 = small.tile([128, NP9], f32)
            r1 = small.tile([128, 1], f32)
            nc.vector.scalar_tensor_tensor(
                out=d1, in0=lnout[:, 0:NP9], scalar=1.0,
                in1=lnout[:, NP9 + RT:2 * NP9 + RT],
                op0=OP.mult, op1=OP.subtract, accum_out=r1[:])
            d2 = small.tile([128, RT], f32)
            r2 = small.tile([128, 1], f32)
            nc.vector.scalar_tensor_tensor(
                out=d2, in0=lnout[:, NP9:NP9 + RT], scalar=1.0,
                in1=lnout[:, 2 * NP9 + RT:2 * NP9 + 2 * RT],
                op0=OP.mult, op1=OP.subtract, accum_out=r2[:])
            part = small.tile([128, 1], f32)
            nc.vector.tensor_sub(part, r1, r2)
            nc.sync.dma_start(out=y_out[:], in_=part[:])

    nc.finalize()
    return nc


def _get_nc():
    if "nc" not in _CACHE:
        _CACHE["nc"] = _build()
    return _CACHE["nc"]


def kernel(s: np.ndarray) -> np.ndarray:
    from concourse.bass_utils import run_bass_kernel_spmd

    s = np.ascontiguousarray(s, dtype=np.float32)
    assert s.shape == (K, D)
    nc = _get_nc()
    in_maps = [
        {"s": np.ascontiguousarray(np.roll(s, -R * c, axis=0))}
        for c in range(NCORES)
    ]
    res = run_bass_kernel_spmd(nc, in_maps, core_ids=list(range(NCORES)))
    _CACHE["last_results"] = res
    total = np.float64(0.0)
    for r in res.results:
        total += np.float64(r["part"].sum(dtype=np.float64))
    return np.array(total / KPOS, dtype=np.float32)

